# revision 9
# baseline (speedup 1.0000x reference)
"""ConvBERT SDConv kernel for Trainium2 (8 NeuronCores, data-parallel over batch).

Problem (per core, batch element b):
  hidden -> depthwise conv (K=9) -> pointwise 768x768 (+bias) -> * query
         -> proj 768->108 (+bias) -> softmax(softmax(.)) over K
  out[s, h, d] = sum_k filt[s, h, k] * value[s + k - 4, h*64 + d]

v3 design notes:
  - everything bf16 on the wire and in matmuls (validated ~6e-3 rel err);
    x/q stream per 512-block (double buffered) so PE starts within ~3us.
  - depthwise diag weight matrices built on-chip: identity (x) per-partition
    scalar on DVE.
  - light conv banded matrices built WITHOUT per-element scatter DMAs:
    per 128-token tile, PE transposes a filt slice ([108, 128] -> [128, 108]),
    then 9 shift-matmuls against constant shifted identities produce
    SHR[s', h, 8-j] = filt[h*9+j, t*120+s'-j].  Each SHR row holds the 9
    band-diagonal values of band row s' for head h CONTIGUOUSLY, so ONE
    rectangular-AP DMA per tile writes the whole band into DRAM with 18-byte
    runs: band[s', h*136 + s' + jr] = SHR[s', h, jr].  Entries with invalid
    s = s' - (8-jr) land in the 8-column pads of each 136-wide head block and
    are never read.  Band zeros are written once (zero-stays-zero); readback
    is one clean rectangular DMA per tile.
  - head matmuls: lhsT = band[:, h*136+8 : h*136+136] (128 cols), moving =
    value rows [t*120, t*120+128) of the padded value.
  - softmax 1/sum via exp(-ln(sum)) on ScalarE (DVE reciprocal is an 8x
    iterative divide, ~3.3us per [12,512] tile).
  - phase-B tiles emitted interleaved after the phase-A block that completes
    their filt range; v loads + out stores issued via gpsimd SWDGE to keep
    the sync HWDGE ring free for the band bounce.
"""

import contextlib

import numpy as np
import ml_dtypes

import concourse.bass as bass
import concourse.bacc as bacc
import concourse.mybir as mybir
import concourse.tile as tile
from concourse.bass_utils import run_bass_kernel_spmd

BF16 = ml_dtypes.bfloat16

# problem constants (hardcoded per contest contract)
B, S, C = 8, 2048, 768
HID = 768
H, K, D = 12, 9, 64
PAD = K // 2                 # 4
NCORES = 8
P = 128                      # partitions
NCH = C // P                 # 6 channel chunks
SB = 512                     # phase-A seq block
NB = S // SB                 # 4
TILE = 120                   # phase-B seq tile (window = TILE + K - 1 = 128)
NT = (S + TILE - 1) // TILE  # 18 (last tile has 8 valid tokens)
HB = TILE + 2 * (K - 1)      # 136: head block width = 8 pad + 120 + 8 pad
BROWP = H * HB               # 1632 band row elements (padded layout)
NSLOT = 6                    # rotating DRAM band buffers
VROWS = PAD + S + TILE       # padded value rows; covers last window
FPAD = 2176                  # filt columns incl zero tail for last tile

F32 = mybir.dt.float32
F32R = mybir.dt.float32r
BF = mybir.dt.bfloat16
AF = mybir.ActivationFunctionType

# phase-B tiles emitted after each phase-A block (tile t reads filt cols
# [t*120, t*120+128), available once block ceil((t*120+128)/512)-1 is done)
TILES_AFTER = {0: [0, 1, 2, 3], 1: [4, 5, 6, 7], 2: [8, 9, 10, 11],
               3: [12, 13, 14, 15, 16, 17]}


def _build_nc(dbg=False):
    nc = bacc.Bacc(
        "TRN2",
        target_bir_lowering=False,
        debug=False,
        enable_asserts=False,
        num_devices=NCORES,
    )
    # per-core inputs
    xT = nc.dram_tensor("xT2", [C, S + 2 * PAD], BF, kind="ExternalInput")
    qT = nc.dram_tensor("qT2", [C, S], BF, kind="ExternalInput")
    vp = nc.dram_tensor("vp2", [VROWS, C], BF, kind="ExternalInput")
    pwT = nc.dram_tensor("pw2", [HID, C], BF, kind="ExternalInput")
    awT = nc.dram_tensor("aw2", [C, H * K], BF, kind="ExternalInput")
    dww = nc.dram_tensor("dww2", [P, NCH * K], F32, kind="ExternalInput")
    bias = nc.dram_tensor("bias2", [C], F32, kind="ExternalInput")
    ab = nc.dram_tensor("ab2", [H * K], F32, kind="ExternalInput")
    e12 = nc.dram_tensor("e12b", [H * K, H], BF, kind="ExternalInput")
    e12t = nc.dram_tensor("e12tr", [H, H * K], F32R, kind="ExternalInput")
    eye = nc.dram_tensor("eye2", [P, P], BF, kind="ExternalInput")
    shf = nc.dram_tensor("shf2", [K, P, P], BF, kind="ExternalInput")
    out = nc.dram_tensor("out", [S, C], BF, kind="ExternalOutput")
    # distinct executable signature per kernel version — the axon/PJRT path has
    # been observed serving a stale compiled executable for same-signature builds
    ver = nc.dram_tensor("ver_salt_v8", [1, 1], F32, kind="ExternalOutput")

    with tile.TileContext(nc) as tc, contextlib.ExitStack() as ctx:
        _kernel_body(tc, ctx, xT, qT, vp, pwT, awT, dww, bias, ab, e12, e12t,
                     eye, shf, out)
        vt = tc.nc.sbuf_tensor([1, 1], F32)
        with vt as vt_t:
            tc.nc.vector.memset(vt_t.ap(), 5.0)
            tc.nc.sync.dma_start(out=ver.ap(), in_=vt_t.ap())

    nc.compile()
    return nc


def _kernel_body(tc, ctx, xT, qT, vp, pwT, awT, dww, bias, ab, e12, e12t,
                 eye, shf, out):
    nc = tc.nc
    add = mybir.AluOpType.add
    mult = mybir.AluOpType.mult
    Exp, Ln = AF.Exp, AF.Ln

    consts = ctx.enter_context(tc.tile_pool(name="consts", bufs=1))
    dramp = ctx.enter_context(tc.tile_pool(name="dramp", bufs=1, space="DRAM"))

    # ---- const loads on the scalar HWDGE ring (sync ring feeds x/q blocks) ----
    # dww + eye first: the depthwise diag build (and so the first dw matmul)
    # depends on them.
    dww_sb = consts.tile([P, NCH * K], F32)
    nc.scalar.dma_start(
        out=dww_sb,
        in_=bass.AP(tensor=dww, offset=0, ap=[[NCH * K, P], [1, NCH * K]]),
    )
    eye_sb = consts.tile([P, P], BF)
    nc.scalar.dma_start(out=eye_sb, in_=eye.ap())
    pw_sb = consts.tile([P, NCH, C], BF)
    nc.scalar.dma_start(
        out=pw_sb,
        in_=bass.AP(tensor=pwT, offset=0, ap=[[C, P], [P * C, NCH], [1, C]]),
    )
    aw_sb = consts.tile([P, NCH, H * K], BF)
    nc.scalar.dma_start(
        out=aw_sb,
        in_=bass.AP(tensor=awT, offset=0, ap=[[H * K, P], [P * H * K, NCH], [1, H * K]]),
    )
    bias_sb = consts.tile([P, NCH], F32)
    nc.scalar.dma_start(
        out=bias_sb, in_=bass.AP(tensor=bias, offset=0, ap=[[1, P], [P, NCH]])
    )
    ab_sb = consts.tile([H * K, 1], F32)
    nc.scalar.dma_start(out=ab_sb, in_=bass.AP(tensor=ab, offset=0, ap=[[1, H * K], [0, 1]]))
    e12_sb = consts.tile([H * K, H], BF)
    nc.scalar.dma_start(out=e12_sb, in_=e12.ap())
    e12t_sb = consts.tile([H, H * K], F32R)
    nc.scalar.dma_start(out=e12t_sb, in_=e12t.ap())
    shf_sb = consts.tile([P, K, P], BF)
    nc.scalar.dma_start(
        out=shf_sb,
        in_=bass.AP(tensor=shf, offset=0, ap=[[P, P], [P * P, K], [1, P]]),
    )

    # full-width double-softmax'd filter, bf16, [108 (h k), FPAD]
    filt_sb = consts.tile([H * K, FPAD], BF)
    nc.vector.memset(filt_sb[:, S:FPAD], 0.0)

    # band bounce buffers in DRAM (rotating), zeroed once via the gpsimd ring
    # (idle at startup); the per-tile diag writes always hit the same
    # positions, so zeros stay zero afterwards.
    zero_sb = consts.tile([P, BROWP], BF)
    nc.vector.memset(zero_sb, 0.0)
    band_drams = []
    zero_insts = []
    for i in range(NSLOT):
        bd = dramp.tile([P, BROWP], BF, tag=f"band{i}")
        zi = nc.gpsimd.dma_start(out=bd, in_=zero_sb)
        band_drams.append(bd)
        zero_insts.append(zi)

    # depthwise diagonal weight matrices, built on-chip (per-chunk tiles so
    # the first dw matmul only waits for its own chunk's 9 builds)
    dwdg_sbs = []
    for c6 in range(NCH):
        dg = consts.tile([P, K, P], BF, tag=f"dwdg{c6}")
        for k in range(K):
            nc.vector.tensor_scalar_mul(
                out=dg[:, k, :], in0=eye_sb, scalar1=dww_sb[:, c6 * K + k:c6 * K + k + 1]
            )
        dwdg_sbs.append(dg)


    # ---- pools ----
    xq = ctx.enter_context(tc.tile_pool(name="xq", bufs=2))
    dwo = ctx.enter_context(tc.tile_pool(name="dwo", bufs=2))
    cap = ctx.enter_context(tc.tile_pool(name="cap", bufs=2))
    smp = ctx.enter_context(tc.tile_pool(name="smp", bufs=2))
    dwps = ctx.enter_context(tc.tile_pool(name="dwps", bufs=1, space="PSUM"))
    pwps = ctx.enter_context(tc.tile_pool(name="pwps", bufs=1, space="PSUM"))
    abps = ctx.enter_context(tc.tile_pool(name="abps", bufs=1, space="PSUM"))
    sups = ctx.enter_context(tc.tile_pool(name="sups", bufs=1, space="PSUM"))
    trps = ctx.enter_context(tc.tile_pool(name="trps", bufs=1, space="PSUM"))
    shps = ctx.enter_context(tc.tile_pool(name="shps", bufs=1, space="PSUM"))
    bopsA = ctx.enter_context(tc.tile_pool(name="bopsA", bufs=1, space="PSUM"))
    bopsB = ctx.enter_context(tc.tile_pool(name="bopsB", bufs=1, space="PSUM"))
    tsp = ctx.enter_context(tc.tile_pool(name="tsp", bufs=2))
    srp = ctx.enter_context(tc.tile_pool(name="srp", bufs=2))
    vtp = ctx.enter_context(tc.tile_pool(name="vtp", bufs=4))
    bsp = ctx.enter_context(tc.tile_pool(name="bsp", bufs=4))
    osp = ctx.enter_context(tc.tile_pool(name="osp", bufs=2))

    prev_readback = {}

    def emit_tile_group(tiles):
        """phase B: banded light conv for a group of <=4 seq tiles.

        The 9 shift matmuls are batched across the group (shared shifted-
        identity stationary, one LDWEIGHTS per j per group)."""
        G = len(tiles)

        # v tile prefetches via gpsimd SWDGE (keeps sync ring free)
        v_sbs = []
        for t in tiles:
            v_sb = vtp.tile([P, C], BF)
            nc.sync.dma_start(out=v_sb, in_=vp.ap()[t * TILE:t * TILE + P, :])
            v_sbs.append(v_sb)

        # filt slice transposes: [108, 128] -> [128, 108] (bf16, PSUM)
        t_sb = tsp.tile([P, G, H, K], BF)
        for g, t in enumerate(tiles):
            t_ps = trps.tile([P, H * K], BF, tag="tr")
            nc.tensor.transpose(t_ps, filt_sb[:, t * TILE:t * TILE + P],
                                eye_sb[0:H * K, 0:H * K])
            nc.vector.tensor_copy(t_sb[:, g], t_ps)

        # 9 shift matmuls over the whole group:
        # SHR[s', g, h, 8-j] = T[s'-j, g, h, j] = filt_g[h*9+j, t*120+s'-j]
        shr_ps = shps.tile([P, G, H, K], F32, tag="sh")
        for j in range(K):
            nc.tensor.matmul(
                shr_ps[:, :, :, K - 1 - j],
                shf_sb[:, j, :],
                t_sb[:, :, :, j],
                start=True, stop=True,
            )
        shr_sb = srp.tile([P, G, H, K], BF)
        nc.vector.tensor_copy(shr_sb, shr_ps)

        for g, t in enumerate(tiles):
            slen = min(TILE, S - t * TILE)
            slot = t % NSLOT
            bd = band_drams[slot]

            # one diag-write DMA: band[s', h*136 + s' + jr] = SHR[s', g, h, jr]
            # (18-byte contiguous runs; invalid entries land in pad columns)
            di = nc.sync.dma_start(
                out=bass.AP(tensor=bd.tensor, offset=bd.offset,
                            ap=[[BROWP + 1, P], [HB, H], [1, K]]),
                in_=shr_sb[:, g],
            )
            war_dep = prev_readback.get(slot, zero_insts[slot])
            tile.add_dep_helper(di.ins, war_dep.ins, reason="band WAR")

            # clean rectangular readback
            band_sb = bsp.tile([P, BROWP], BF)
            rb = nc.gpsimd.dma_start(
                out=band_sb,
                in_=bass.AP(tensor=bd.tensor, offset=bd.offset,
                            ap=[[BROWP, P], [1, BROWP]]),
            )
            tile.add_dep_helper(rb.ins, di.ins, reason="band RAW")
            prev_readback[slot] = rb

            v_sb = v_sbs[g]
            o_sb = osp.tile([P, C], BF)
            for half, pool in ((0, bopsA), (1, bopsB)):
                ops = pool.tile([P, C // 2], F32)
                for hh in range(H // 2):
                    h = half * (H // 2) + hh
                    nc.tensor.matmul(
                        ops[:, hh * D:(hh + 1) * D],
                        band_sb[:, h * HB + K - 1:h * HB + K - 1 + P],
                        v_sb[:, h * D:(h + 1) * D],
                        start=True, stop=True,
                    )
                nc.scalar.copy(out=o_sb[:slen, half * (C // 2):(half + 1) * (C // 2)],
                               in_=ops[:slen])
            nc.gpsimd.dma_start(out=out.ap()[t * TILE:t * TILE + slen, :],
                                in_=o_sb[:slen])

    # ---------------- phase A (with interleaved phase-B tiles) ----------------
    def load_xq(b):
        s0 = b * SB
        x_blk = xq.tile([P, NCH, SB + 2 * PAD], BF, tag=f"x{b % 2}")
        nc.sync.dma_start(
            out=x_blk,
            in_=bass.AP(
                tensor=xT, offset=s0,
                ap=[[S + 2 * PAD, P], [P * (S + 2 * PAD), NCH], [1, SB + 2 * PAD]],
            ),
        )
        q_blk = xq.tile([P, NCH, SB], BF, tag=f"q{b % 2}")
        nc.sync.dma_start(
            out=q_blk,
            in_=bass.AP(tensor=qT, offset=s0, ap=[[S, P], [P * S, NCH], [1, SB]]),
        )
        return x_blk, q_blk

    xq_tiles = {0: load_xq(0)}
    for b in range(NB):
        s0 = b * SB
        if b + 1 < NB:
            xq_tiles[b + 1] = load_xq(b + 1)
        x_blk, q_blk = xq_tiles.pop(b)

        # depthwise conv: 9 accumulating diagonal matmuls per channel chunk
        dw_blk = dwo.tile([P, NCH, SB], BF)
        for c6 in range(NCH):
            dps = dwps.tile([P, SB], F32, tag="dw")
            for k in range(K):
                nc.tensor.matmul(
                    dps,
                    dwdg_sbs[c6][:, k, :],
                    x_blk[:, c6, k:k + SB],
                    start=(k == 0), stop=(k == K - 1),
                )
            nc.scalar.copy(out=dw_blk[:, c6, :], in_=dps)

        # pointwise matmul + fused (bias add, * query) evacuation -> bf16
        ca_blk = cap.tile([P, NCH, SB], BF)
        for cc in range(NCH):
            pps = pwps.tile([P, SB], F32, tag="pw")
            for hc in range(NCH):
                nc.tensor.matmul(
                    pps,
                    pw_sb[:, hc, cc * P:(cc + 1) * P],
                    dw_blk[:, hc, :],
                    start=(hc == 0), stop=(hc == NCH - 1),
                )
            nc.vector.scalar_tensor_tensor(
                out=ca_blk[:, cc, :],
                in0=pps,
                scalar=bias_sb[:, cc:cc + 1],
                in1=q_blk[:, cc, :],
                op0=add, op1=mult,
            )

        # projection to [108, SB]
        aps = abps.tile([H * K, SB], F32, tag="ab")
        for cc in range(NCH):
            nc.tensor.matmul(
                aps,
                aw_sb[:, cc, :],
                ca_blk[:, cc, :],
                start=(cc == 0), stop=(cc == NCH - 1),
            )

        # double softmax over k; 1/sum computed as exp(-ln(sum))
        u1 = smp.tile([H * K, SB], BF, tag="u1")
        nc.scalar.activation(out=u1, in_=aps, func=Exp, bias=ab_sb, scale=1.0)
        s1 = sups.tile([H, SB], F32, tag="sum")
        nc.tensor.matmul(s1, e12_sb[:], u1[:], start=True, stop=True)
        lt1 = smp.tile([H, SB], F32R, tag="lt")
        with nc.allow_low_precision(reason="f32r holds full fp32 bits"):
            nc.scalar.activation(out=lt1, in_=s1, func=Ln)
        b1 = abps.tile([H * K, SB], F32, tag="ab")
        nc.tensor.matmul(b1, e12t_sb[:], lt1[:], start=True, stop=True)
        bx1 = smp.tile([H * K, SB], BF, tag="bx")
        nc.scalar.activation(out=bx1, in_=b1, func=Exp, scale=-1.0)
        p1 = smp.tile([H * K, SB], BF, tag="p1")
        nc.vector.tensor_mul(out=p1, in0=u1, in1=bx1)

        u2 = smp.tile([H * K, SB], BF, tag="u2")
        nc.scalar.activation(out=u2, in_=p1, func=Exp)
        s2 = sups.tile([H, SB], F32, tag="sum")
        nc.tensor.matmul(s2, e12_sb[:], u2[:], start=True, stop=True)
        lt2 = smp.tile([H, SB], F32R, tag="lt")
        with nc.allow_low_precision(reason="f32r holds full fp32 bits"):
            nc.scalar.activation(out=lt2, in_=s2, func=Ln)
        b2 = abps.tile([H * K, SB], F32, tag="ab")
        nc.tensor.matmul(b2, e12t_sb[:], lt2[:], start=True, stop=True)
        bx2 = smp.tile([H * K, SB], BF, tag="bx")
        nc.scalar.activation(out=bx2, in_=b2, func=Exp, scale=-1.0)
        nc.vector.tensor_mul(out=filt_sb[:, s0:s0 + SB], in0=u2, in1=bx2)

        tiles = TILES_AFTER[b]
        gsz = 4 if b < NB - 1 else 2
        for i in range(0, len(tiles), gsz):
            emit_tile_group(tiles[i:i + gsz])


_NC_CACHE = {}


def get_nc(dbg=False):
    if dbg not in _NC_CACHE:
        _NC_CACHE[dbg] = _build_nc(dbg)
    return _NC_CACHE[dbg]


def make_in_maps(query, value, hidden_states, dw_weight, pw_weight, sep_bias,
                 attn_W, attn_b):
    query = np.asarray(query, np.float32)
    value = np.asarray(value, np.float32)
    hidden_states = np.asarray(hidden_states, np.float32)
    dw_weight = np.asarray(dw_weight, np.float32)
    pw_weight = np.asarray(pw_weight, np.float32)
    sep_bias = np.asarray(sep_bias, np.float32)
    attn_W = np.asarray(attn_W, np.float32)
    attn_b = np.asarray(attn_b, np.float32)

    # shared (weight) tensors
    dww = np.ascontiguousarray(
        dw_weight[:, 0, :].reshape(NCH, P, K).transpose(1, 0, 2).reshape(P, NCH * K)
    )
    pwT = np.ascontiguousarray(pw_weight[:, :, 0].T).astype(BF16)
    awT = np.ascontiguousarray(attn_W.T).astype(BF16)
    e12 = np.repeat(np.eye(H, dtype=np.float32), K, axis=0)  # [108, 12]
    e12b = e12.astype(BF16)
    e12tr = np.ascontiguousarray(e12.T)
    eye = np.eye(P, dtype=np.float32).astype(BF16)
    shfm = np.stack([np.eye(P, P, k=j, dtype=np.float32) for j in range(K)])
    shfm = shfm.astype(BF16)

    in_maps = []
    for b in range(NCORES):
        xTb = np.zeros((C, S + 2 * PAD), BF16)
        xTb[:, PAD:PAD + S] = hidden_states[b].T.astype(BF16)
        qTb = np.ascontiguousarray(query[b].T).astype(BF16)
        vpad = np.zeros((VROWS, C), BF16)
        vpad[PAD:PAD + S] = value[b].astype(BF16)
        in_maps.append({
            "xT2": xTb, "qT2": qTb, "vp2": vpad,
            "pw2": pwT, "aw2": awT, "dww2": dww,
            "bias2": sep_bias, "ab2": attn_b,
            "e12b": e12b, "e12tr": e12tr, "eye2": eye, "shf2": shfm,
        })
    return in_maps


def kernel(query, value, hidden_states, dw_weight, pw_weight, sep_bias,
           attn_W, attn_b, num_heads=None, kernel_size=None):
    nc = get_nc()
    in_maps = make_in_maps(query, value, hidden_states, dw_weight, pw_weight,
                           sep_bias, attn_W, attn_b)
    res = run_bass_kernel_spmd(nc, in_maps, core_ids=list(range(NCORES)))
    outs = [np.asarray(r["out"]).astype(np.float32) for r in res.results]
    return np.stack(outs, axis=0).reshape(B, S, H, D)


# revision 10
# speedup vs baseline: 1.0495x; 1.0495x over previous
"""ConvBERT SDConv kernel for Trainium2 (8 NeuronCores, data-parallel over batch).

Problem (per core, batch element b):
  hidden -> depthwise conv (K=9) -> pointwise 768x768 (+bias) -> * query
         -> proj 768->108 (+bias) -> softmax(softmax(.)) over K
  out[s, h, d] = sum_k filt[s, h, k] * value[s + k - 4, h*64 + d]

v3 design notes:
  - everything bf16 on the wire and in matmuls (validated ~6e-3 rel err);
    x/q stream per 512-block (double buffered) so PE starts within ~3us.
  - depthwise diag weight matrices built on-chip: identity (x) per-partition
    scalar on DVE.
  - light conv banded matrices built WITHOUT per-element scatter DMAs:
    per 128-token tile, PE transposes a filt slice ([108, 128] -> [128, 108]),
    then 9 shift-matmuls against constant shifted identities produce
    SHR[s', h, 8-j] = filt[h*9+j, t*120+s'-j].  Each SHR row holds the 9
    band-diagonal values of band row s' for head h CONTIGUOUSLY, so ONE
    rectangular-AP DMA per tile writes the whole band into DRAM with 18-byte
    runs: band[s', h*136 + s' + jr] = SHR[s', h, jr].  Entries with invalid
    s = s' - (8-jr) land in the 8-column pads of each 136-wide head block and
    are never read.  Band zeros are written once (zero-stays-zero); readback
    is one clean rectangular DMA per tile.
  - head matmuls: lhsT = band[:, h*136+8 : h*136+136] (128 cols), moving =
    value rows [t*120, t*120+128) of the padded value.
  - softmax 1/sum via exp(-ln(sum)) on ScalarE (DVE reciprocal is an 8x
    iterative divide, ~3.3us per [12,512] tile).
  - phase-B tiles emitted interleaved after the phase-A block that completes
    their filt range; v loads + out stores issued via gpsimd SWDGE to keep
    the sync HWDGE ring free for the band bounce.
"""

import contextlib

import numpy as np
import ml_dtypes

import concourse.bass as bass
import concourse.bacc as bacc
import concourse.mybir as mybir
import concourse.tile as tile
from concourse.bass_utils import run_bass_kernel_spmd

BF16 = ml_dtypes.bfloat16

# problem constants (hardcoded per contest contract)
B, S, C = 8, 2048, 768
HID = 768
H, K, D = 12, 9, 64
PAD = K // 2                 # 4
NCORES = 8
P = 128                      # partitions
NCH = C // P                 # 6 channel chunks
SB = 512                     # phase-A seq block
NB = S // SB                 # 4
TILE = 120                   # phase-B seq tile (window = TILE + K - 1 = 128)
NT = (S + TILE - 1) // TILE  # 18 (last tile has 8 valid tokens)
HB = TILE + 2 * (K - 1)      # 136: head block width = 8 pad + 120 + 8 pad
BROWP = H * HB               # 1632 band row elements (padded layout)
NSLOT = 6                    # rotating DRAM band buffers
VROWS = PAD + S + TILE       # padded value rows; covers last window
FPAD = 2176                  # filt columns incl zero tail for last tile

F32 = mybir.dt.float32
F32R = mybir.dt.float32r
BF = mybir.dt.bfloat16
AF = mybir.ActivationFunctionType

# phase-B tiles emitted after each phase-A block (tile t reads filt cols
# [t*120, t*120+128), available once block ceil((t*120+128)/512)-1 is done)
TILES_AFTER = {0: [0, 1, 2, 3], 1: [4, 5, 6, 7], 2: [8, 9, 10, 11],
               3: [12, 13, 14, 15, 16, 17]}


def _build_nc(dbg=False):
    nc = bacc.Bacc(
        "TRN2",
        target_bir_lowering=False,
        debug=False,
        enable_asserts=False,
        num_devices=NCORES,
    )
    # per-core inputs
    xT = nc.dram_tensor("xT2", [C, S + 2 * PAD], BF, kind="ExternalInput")
    qT = nc.dram_tensor("qT2", [C, S], BF, kind="ExternalInput")
    vp = nc.dram_tensor("vp2", [VROWS, C], BF, kind="ExternalInput")
    pwT = nc.dram_tensor("pw2", [HID, C], BF, kind="ExternalInput")
    awT = nc.dram_tensor("aw2", [C, H * K], BF, kind="ExternalInput")
    dww = nc.dram_tensor("dww2", [P, NCH * K], F32, kind="ExternalInput")
    bias = nc.dram_tensor("bias2", [C], F32, kind="ExternalInput")
    ab = nc.dram_tensor("ab2", [H * K], F32, kind="ExternalInput")
    e12 = nc.dram_tensor("e12b", [H * K, H], BF, kind="ExternalInput")
    e12t = nc.dram_tensor("e12tr", [H, H * K], F32R, kind="ExternalInput")
    eye = nc.dram_tensor("eye2", [P, P], BF, kind="ExternalInput")
    shf = nc.dram_tensor("shf2", [K, P, P], BF, kind="ExternalInput")
    out = nc.dram_tensor("out", [S, C], BF, kind="ExternalOutput")
    # distinct executable signature per kernel version — the axon/PJRT path has
    # been observed serving a stale compiled executable for same-signature builds
    ver = nc.dram_tensor("ver_salt_v9", [1, 1], F32, kind="ExternalOutput")

    with tile.TileContext(nc) as tc, contextlib.ExitStack() as ctx:
        _kernel_body(tc, ctx, xT, qT, vp, pwT, awT, dww, bias, ab, e12, e12t,
                     eye, shf, out)
        vt = tc.nc.sbuf_tensor([1, 1], F32)
        with vt as vt_t:
            tc.nc.vector.memset(vt_t.ap(), 5.0)
            tc.nc.sync.dma_start(out=ver.ap(), in_=vt_t.ap())

    nc.compile()
    return nc


def _kernel_body(tc, ctx, xT, qT, vp, pwT, awT, dww, bias, ab, e12, e12t,
                 eye, shf, out):
    nc = tc.nc
    add = mybir.AluOpType.add
    mult = mybir.AluOpType.mult
    Exp, Ln = AF.Exp, AF.Ln

    consts = ctx.enter_context(tc.tile_pool(name="consts", bufs=1))
    dramp = ctx.enter_context(tc.tile_pool(name="dramp", bufs=1, space="DRAM"))

    # ---- const loads on the scalar HWDGE ring (sync ring feeds x/q blocks) ----
    # dww + eye first: the depthwise diag build (and so the first dw matmul)
    # depends on them.
    dww_sb = consts.tile([P, NCH * K], F32)
    nc.scalar.dma_start(
        out=dww_sb,
        in_=bass.AP(tensor=dww, offset=0, ap=[[NCH * K, P], [1, NCH * K]]),
    )
    eye_sb = consts.tile([P, P], BF)
    nc.scalar.dma_start(out=eye_sb, in_=eye.ap())
    pw_sb = consts.tile([P, NCH, C], BF)
    nc.scalar.dma_start(
        out=pw_sb,
        in_=bass.AP(tensor=pwT, offset=0, ap=[[C, P], [P * C, NCH], [1, C]]),
    )
    aw_sb = consts.tile([P, NCH, H * K], BF)
    nc.scalar.dma_start(
        out=aw_sb,
        in_=bass.AP(tensor=awT, offset=0, ap=[[H * K, P], [P * H * K, NCH], [1, H * K]]),
    )
    bias_sb = consts.tile([P, NCH], F32)
    nc.scalar.dma_start(
        out=bias_sb, in_=bass.AP(tensor=bias, offset=0, ap=[[1, P], [P, NCH]])
    )
    ab_sb = consts.tile([H * K, 1], F32)
    nc.scalar.dma_start(out=ab_sb, in_=bass.AP(tensor=ab, offset=0, ap=[[1, H * K], [0, 1]]))
    e12_sb = consts.tile([H * K, H], BF)
    nc.scalar.dma_start(out=e12_sb, in_=e12.ap())
    e12t_sb = consts.tile([H, H * K], F32R)
    nc.scalar.dma_start(out=e12t_sb, in_=e12t.ap())
    shf_sb = consts.tile([P, K, P], BF)
    nc.scalar.dma_start(
        out=shf_sb,
        in_=bass.AP(tensor=shf, offset=0, ap=[[P, P], [P * P, K], [1, P]]),
    )

    # full-width double-softmax'd filter, bf16, [108 (h k), FPAD]
    filt_sb = consts.tile([H * K, FPAD], BF)
    nc.vector.memset(filt_sb[:, S:FPAD], 0.0)

    # band bounce buffers in DRAM (rotating), zeroed once via the gpsimd ring
    # (idle at startup); the per-tile diag writes always hit the same
    # positions, so zeros stay zero afterwards.
    zero_sb = consts.tile([P, BROWP], BF)
    nc.vector.memset(zero_sb, 0.0)
    band_drams = []
    zero_insts = []
    for i in range(NSLOT):
        bd = dramp.tile([P, BROWP], BF, tag=f"band{i}")
        zi = nc.scalar.dma_start(out=bd, in_=zero_sb)
        band_drams.append(bd)
        zero_insts.append(zi)

    # depthwise diagonal weight matrices, built on-chip (per-chunk tiles so
    # the first dw matmul only waits for its own chunk's 9 builds)
    dwdg_sbs = []
    for c6 in range(NCH):
        dg = consts.tile([P, K, P], BF, tag=f"dwdg{c6}")
        for k in range(K):
            nc.vector.tensor_scalar_mul(
                out=dg[:, k, :], in0=eye_sb, scalar1=dww_sb[:, c6 * K + k:c6 * K + k + 1]
            )
        dwdg_sbs.append(dg)


    # ---- pools ----
    xq = ctx.enter_context(tc.tile_pool(name="xq", bufs=2))
    dwo = ctx.enter_context(tc.tile_pool(name="dwo", bufs=2))
    cap = ctx.enter_context(tc.tile_pool(name="cap", bufs=2))
    smp = ctx.enter_context(tc.tile_pool(name="smp", bufs=2))
    dwps = ctx.enter_context(tc.tile_pool(name="dwps", bufs=1, space="PSUM"))
    pwps = ctx.enter_context(tc.tile_pool(name="pwps", bufs=1, space="PSUM"))
    abps = ctx.enter_context(tc.tile_pool(name="abps", bufs=1, space="PSUM"))
    sups = ctx.enter_context(tc.tile_pool(name="sups", bufs=1, space="PSUM"))
    trps = ctx.enter_context(tc.tile_pool(name="trps", bufs=1, space="PSUM"))
    shps = ctx.enter_context(tc.tile_pool(name="shps", bufs=1, space="PSUM"))
    bopsA = ctx.enter_context(tc.tile_pool(name="bopsA", bufs=1, space="PSUM"))
    bopsB = ctx.enter_context(tc.tile_pool(name="bopsB", bufs=1, space="PSUM"))
    tsp = ctx.enter_context(tc.tile_pool(name="tsp", bufs=2))
    srp = ctx.enter_context(tc.tile_pool(name="srp", bufs=2))
    vtp = ctx.enter_context(tc.tile_pool(name="vtp", bufs=10))
    bsp = ctx.enter_context(tc.tile_pool(name="bsp", bufs=10))
    osp = ctx.enter_context(tc.tile_pool(name="osp", bufs=2))

    prev_readback = {}

    def band_prep(tiles):
        """phase B part 1: build + bounce the banded matrices for <=4 tiles.

        PE cost is small (transposes + shift matmuls); the band DMA roundtrip
        completes while the NEXT phase-A block's matmuls run.  Returns the
        per-tile state consumed by head_part one block later."""
        G = len(tiles)

        state = []
        # v tile prefetches (sync ring; consumed a block later)
        for t in tiles:
            v_sb = vtp.tile([P, C], BF)
            nc.sync.dma_start(out=v_sb, in_=vp.ap()[t * TILE:t * TILE + P, :])
            state.append({"t": t, "v": v_sb})

        # filt slice transposes: [108, 128] -> [128, 108] (bf16, PSUM)
        t_sb = tsp.tile([P, G, H, K], BF)
        for g, t in enumerate(tiles):
            t_ps = trps.tile([P, H * K], BF, tag="tr")
            nc.tensor.transpose(t_ps, filt_sb[:, t * TILE:t * TILE + P],
                                eye_sb[0:H * K, 0:H * K])
            nc.vector.tensor_copy(t_sb[:, g], t_ps)

        # 9 shift matmuls over the whole group:
        # SHR[s', g, h, 8-j] = T[s'-j, g, h, j] = filt_g[h*9+j, t*120+s'-j]
        shr_ps = shps.tile([P, G, H, K], F32, tag="sh")
        for j in range(K):
            nc.tensor.matmul(
                shr_ps[:, :, :, K - 1 - j],
                shf_sb[:, j, :],
                t_sb[:, :, :, j],
                start=True, stop=True,
            )
        shr_sb = srp.tile([P, G, H, K], BF)
        nc.vector.tensor_copy(shr_sb, shr_ps)

        for g, t in enumerate(tiles):
            slot = t % NSLOT
            bd = band_drams[slot]

            # one diag-write DMA: band[s', h*136 + s' + jr] = SHR[s', g, h, jr]
            # (18-byte contiguous runs; invalid entries land in pad columns)
            di = nc.sync.dma_start(
                out=bass.AP(tensor=bd.tensor, offset=bd.offset,
                            ap=[[BROWP + 1, P], [HB, H], [1, K]]),
                in_=shr_sb[:, g],
            )
            war_dep = prev_readback.get(slot, zero_insts[slot])
            tile.add_dep_helper(di.ins, war_dep.ins, reason="band WAR")

            # clean rectangular readback
            band_sb = bsp.tile([P, BROWP], BF)
            rb = nc.sync.dma_start(
                out=band_sb,
                in_=bass.AP(tensor=bd.tensor, offset=bd.offset,
                            ap=[[BROWP, P], [1, BROWP]]),
            )
            tile.add_dep_helper(rb.ins, di.ins, reason="band RAW")
            prev_readback[slot] = rb
            state[g]["band"] = band_sb
        return state

    def head_part(state):
        """phase B part 2: per-head banded matmuls + output store."""
        for st in state:
            t, v_sb, band_sb = st["t"], st["v"], st["band"]
            slen = min(TILE, S - t * TILE)
            o_sb = osp.tile([P, C], BF)
            for half, pool in ((0, bopsA), (1, bopsB)):
                ops = pool.tile([P, C // 2], F32)
                for hh in range(H // 2):
                    h = half * (H // 2) + hh
                    nc.tensor.matmul(
                        ops[:, hh * D:(hh + 1) * D],
                        band_sb[:, h * HB + K - 1:h * HB + K - 1 + P],
                        v_sb[:, h * D:(h + 1) * D],
                        start=True, stop=True,
                    )
                nc.scalar.copy(out=o_sb[:slen, half * (C // 2):(half + 1) * (C // 2)],
                               in_=ops[:slen])
            nc.scalar.dma_start(out=out.ap()[t * TILE:t * TILE + slen, :],
                                in_=o_sb[:slen])

    # ---------------- phase A (with interleaved phase-B tiles) ----------------
    def load_xq(b):
        s0 = b * SB
        x_blk = xq.tile([P, NCH, SB + 2 * PAD], BF, tag=f"x{b % 2}")
        nc.sync.dma_start(
            out=x_blk,
            in_=bass.AP(
                tensor=xT, offset=s0,
                ap=[[S + 2 * PAD, P], [P * (S + 2 * PAD), NCH], [1, SB + 2 * PAD]],
            ),
        )
        q_blk = xq.tile([P, NCH, SB], BF, tag=f"q{b % 2}")
        nc.sync.dma_start(
            out=q_blk,
            in_=bass.AP(tensor=qT, offset=s0, ap=[[S, P], [P * S, NCH], [1, SB]]),
        )
        return x_blk, q_blk

    xq_tiles = {0: load_xq(0)}
    pending = []
    for b in range(NB):
        s0 = b * SB
        if b + 1 < NB:
            xq_tiles[b + 1] = load_xq(b + 1)
        x_blk, q_blk = xq_tiles.pop(b)

        # depthwise conv: 9 accumulating diagonal matmuls per channel chunk
        dw_blk = dwo.tile([P, NCH, SB], BF)
        for c6 in range(NCH):
            dps = dwps.tile([P, SB], F32, tag="dw")
            for k in range(K):
                nc.tensor.matmul(
                    dps,
                    dwdg_sbs[c6][:, k, :],
                    x_blk[:, c6, k:k + SB],
                    start=(k == 0), stop=(k == K - 1),
                )
            nc.scalar.copy(out=dw_blk[:, c6, :], in_=dps)

        # pointwise matmul + fused (bias add, * query) evacuation -> bf16
        ca_blk = cap.tile([P, NCH, SB], BF)
        for cc in range(NCH):
            pps = pwps.tile([P, SB], F32, tag="pw")
            for hc in range(NCH):
                nc.tensor.matmul(
                    pps,
                    pw_sb[:, hc, cc * P:(cc + 1) * P],
                    dw_blk[:, hc, :],
                    start=(hc == 0), stop=(hc == NCH - 1),
                )
            nc.vector.scalar_tensor_tensor(
                out=ca_blk[:, cc, :],
                in0=pps,
                scalar=bias_sb[:, cc:cc + 1],
                in1=q_blk[:, cc, :],
                op0=add, op1=mult,
            )

        # projection to [108, SB]
        aps = abps.tile([H * K, SB], F32, tag="ab")
        for cc in range(NCH):
            nc.tensor.matmul(
                aps,
                aw_sb[:, cc, :],
                ca_blk[:, cc, :],
                start=(cc == 0), stop=(cc == NCH - 1),
            )

        # double softmax over k; 1/sum computed as exp(-ln(sum))
        u1 = smp.tile([H * K, SB], BF, tag="u1")
        nc.scalar.activation(out=u1, in_=aps, func=Exp, bias=ab_sb, scale=1.0)
        s1 = sups.tile([H, SB], F32, tag="sum")
        nc.tensor.matmul(s1, e12_sb[:], u1[:], start=True, stop=True)
        lt1 = smp.tile([H, SB], F32R, tag="lt")
        with nc.allow_low_precision(reason="f32r holds full fp32 bits"):
            nc.scalar.activation(out=lt1, in_=s1, func=Ln)
        b1 = abps.tile([H * K, SB], F32, tag="ab")
        nc.tensor.matmul(b1, e12t_sb[:], lt1[:], start=True, stop=True)
        bx1 = smp.tile([H * K, SB], BF, tag="bx")
        nc.scalar.activation(out=bx1, in_=b1, func=Exp, scale=-1.0)
        p1 = smp.tile([H * K, SB], BF, tag="p1")
        nc.vector.tensor_mul(out=p1, in0=u1, in1=bx1)

        u2 = smp.tile([H * K, SB], BF, tag="u2")
        nc.scalar.activation(out=u2, in_=p1, func=Exp)
        s2 = sups.tile([H, SB], F32, tag="sum")
        nc.tensor.matmul(s2, e12_sb[:], u2[:], start=True, stop=True)
        lt2 = smp.tile([H, SB], F32R, tag="lt")
        with nc.allow_low_precision(reason="f32r holds full fp32 bits"):
            nc.scalar.activation(out=lt2, in_=s2, func=Ln)
        b2 = abps.tile([H * K, SB], F32, tag="ab")
        nc.tensor.matmul(b2, e12t_sb[:], lt2[:], start=True, stop=True)
        bx2 = smp.tile([H * K, SB], BF, tag="bx")
        nc.scalar.activation(out=bx2, in_=b2, func=Exp, scale=-1.0)
        nc.vector.tensor_mul(out=filt_sb[:, s0:s0 + SB], in0=u2, in1=bx2)

        while pending:
            head_part(pending.pop(0))
        tiles = TILES_AFTER[b]
        gsz = 4 if b < NB - 1 else 3
        for i in range(0, len(tiles), gsz):
            pending.append(band_prep(tiles[i:i + gsz]))

    while pending:
        head_part(pending.pop(0))


_NC_CACHE = {}


def get_nc(dbg=False):
    if dbg not in _NC_CACHE:
        _NC_CACHE[dbg] = _build_nc(dbg)
    return _NC_CACHE[dbg]


def make_in_maps(query, value, hidden_states, dw_weight, pw_weight, sep_bias,
                 attn_W, attn_b):
    query = np.asarray(query, np.float32)
    value = np.asarray(value, np.float32)
    hidden_states = np.asarray(hidden_states, np.float32)
    dw_weight = np.asarray(dw_weight, np.float32)
    pw_weight = np.asarray(pw_weight, np.float32)
    sep_bias = np.asarray(sep_bias, np.float32)
    attn_W = np.asarray(attn_W, np.float32)
    attn_b = np.asarray(attn_b, np.float32)

    # shared (weight) tensors
    dww = np.ascontiguousarray(
        dw_weight[:, 0, :].reshape(NCH, P, K).transpose(1, 0, 2).reshape(P, NCH * K)
    )
    pwT = np.ascontiguousarray(pw_weight[:, :, 0].T).astype(BF16)
    awT = np.ascontiguousarray(attn_W.T).astype(BF16)
    e12 = np.repeat(np.eye(H, dtype=np.float32), K, axis=0)  # [108, 12]
    e12b = e12.astype(BF16)
    e12tr = np.ascontiguousarray(e12.T)
    eye = np.eye(P, dtype=np.float32).astype(BF16)
    shfm = np.stack([np.eye(P, P, k=j, dtype=np.float32) for j in range(K)])
    shfm = shfm.astype(BF16)

    in_maps = []
    for b in range(NCORES):
        xTb = np.zeros((C, S + 2 * PAD), BF16)
        xTb[:, PAD:PAD + S] = hidden_states[b].T.astype(BF16)
        qTb = np.ascontiguousarray(query[b].T).astype(BF16)
        vpad = np.zeros((VROWS, C), BF16)
        vpad[PAD:PAD + S] = value[b].astype(BF16)
        in_maps.append({
            "xT2": xTb, "qT2": qTb, "vp2": vpad,
            "pw2": pwT, "aw2": awT, "dww2": dww,
            "bias2": sep_bias, "ab2": attn_b,
            "e12b": e12b, "e12tr": e12tr, "eye2": eye, "shf2": shfm,
        })
    return in_maps


def kernel(query, value, hidden_states, dw_weight, pw_weight, sep_bias,
           attn_W, attn_b, num_heads=None, kernel_size=None):
    nc = get_nc()
    in_maps = make_in_maps(query, value, hidden_states, dw_weight, pw_weight,
                           sep_bias, attn_W, attn_b)
    res = run_bass_kernel_spmd(nc, in_maps, core_ids=list(range(NCORES)))
    outs = [np.asarray(r["out"]).astype(np.float32) for r in res.results]
    return np.stack(outs, axis=0).reshape(B, S, H, D)
